# Initial kernel scaffold
#
"""Deformable-conv layer kernel for 8 Trainium2 NeuronCores (Bass/Tile).

kernel(**inputs): full inputs -> full output [2,48,48,24] f32.
Data parallel over (batch, H/4) -> 8 shards of 576 pixels.

Per core (576 pixels s, 216 sample-channels k):
  G[y,k,c]  = sum_ch M[y*48+c, ch] * k4[k, ch]       (PE 1x1-conv compress)
  one stream_shuffle replicates y_k to rows 0-63 and x_k to rows 64-127,
  one Abs + one Relu build both tent (bilinear-weight) matrices:
  Ry[r, s]  = tent(y_k[s] - r), Cx[c, s] = tent(x_k[s] - c)
  T_k       = G_k^T @ Ry   -> [48c x 576s]           (PE, N=512+64 split)
  P_k       = T_k * Cx_k                             (DVE, one op)
  out_u    += ones^T @ P_k  (partition sum + 9-tap PSUM accumulation)
tent(d) = relu(1-|d|) equals the reference bilinear weights exactly; the
y==47 / x==47 clip corner (reference weights all zero there) is handled
by adding 1e6 to the coordinate so the tents vanish.
"""

import sys

for _p in ("/opt/trn_rl_repo",):
    if _p not in sys.path:
        sys.path.insert(0, _p)

import numpy as np

B, H, W, C = 2, 48, 48, 32
U = 24
KH = KW = 3
PAD = 1
K = KH * KW * U          # 216
NCORES = 8
HLOC = H // 4            # 12
S = HLOC * W             # 576
DY = 48                  # y domain (full; border samples have base 0)
DX = 48                  # x domain
BIG = 1.0e6

_PROG = {}


def _base_grids():
    hh = np.arange(H)[:, None, None, None] + np.arange(KH)[None, None, :, None] - PAD
    ww = np.arange(W)[None, :, None, None] + np.arange(KW)[None, None, None, :] - PAD
    hh = np.broadcast_to(hh, (H, W, KH, KW))
    ww = np.broadcast_to(ww, (H, W, KH, KW))
    valid = (hh >= 0) & (hh < H) & (ww >= 0) & (ww < W)
    yb = np.where(valid, hh, 0).reshape(H, W, 9).astype(np.float32)
    xb = np.where(valid, ww, 0).reshape(H, W, 9).astype(np.float32)
    return yb, xb


def _build_program():
    import concourse.mybir as mybir
    import concourse.tile as tile
    from concourse import bacc

    f32 = mybir.dt.float32
    Alu = mybir.AluOpType
    Act = mybir.ActivationFunctionType

    nc = bacc.Bacc("TRN2", target_bir_lowering=False, debug=False)

    d_mtw = nc.declare_dram_parameter("mtw", [C, H * W], f32, isOutput=False)
    d_kct = nc.declare_dram_parameter("kct", [C, K], f32, isOutput=False)
    d_offy = nc.declare_dram_parameter("offy", [K, S], f32, isOutput=False)
    d_offx = nc.declare_dram_parameter("offx", [K, S], f32, isOutput=False)
    d_ybf = nc.declare_dram_parameter("ybf", [K, S], f32, isOutput=False)
    d_xbf = nc.declare_dram_parameter("xbf", [K, S], f32, isOutput=False)
    # consts [128, 1]: -iota (rows 0-63; junk above 48 is harmless)
    d_cst = nc.declare_dram_parameter("cst", [128, 1], f32, isOutput=False)
    d_ones = nc.declare_dram_parameter("ones48", [48, 1], f32, isOutput=False)
    d_bias = nc.declare_dram_parameter("biasr", [1, U], f32, isOutput=False)
    d_out = nc.declare_dram_parameter("out", [1, U * S], f32, isOutput=True)

    NB = (K + 31) // 32                 # 7 blocks of 32 k
    NT = (K + 63) // 64                 # 4 prep tiles of 64 k

    with tile.TileContext(nc) as tc:
        with (
            tc.tile_pool(name="persist", bufs=1) as pp,
            tc.tile_pool(name="work", bufs=3) as wp,
            tc.tile_pool(name="psA", bufs=2, space="PSUM") as psA,
            tc.tile_pool(name="psB", bufs=1, space="PSUM") as psB,
            tc.tile_pool(name="psG", bufs=1, space="PSUM") as psG,
        ):
            mtw0 = pp.tile([C, H * W], f32, tag="mtw0", name="mtw0")
            nc.sync.dma_start(out=mtw0[:], in_=d_mtw[:])
            kct0 = pp.tile([C, K], f32, tag="kct0", name="kct0")
            nc.sync.dma_start(out=kct0[:], in_=d_kct[:])
            # PE operands go through DVE copies so Matmult instructions
            # wait on engine semaphores only (walrus chokes on multiple
            # DMA-lane waits attached to one LdWeights).
            mtw = pp.tile([C, H * W], f32, tag="mtw", name="mtw")
            nc.vector.tensor_copy(out=mtw[:], in_=mtw0[:])
            kct = pp.tile([C, K], f32, tag="kct", name="kct")
            nc.vector.tensor_copy(out=kct[:], in_=kct0[:])
            cst = pp.tile([128, 1], f32, tag="cst", name="cst")
            nc.sync.dma_start(out=cst[:], in_=d_cst[:])
            ones0 = pp.tile([48, 1], f32, tag="ones0", name="ones0")
            nc.sync.dma_start(out=ones0[:], in_=d_ones[:])
            ones48 = pp.tile([48, 1], f32, tag="ones48", name="ones48")
            nc.vector.tensor_copy(out=ones48[:], in_=ones0[:])
            biasr = pp.tile([1, U], f32, tag="biasr", name="biasr")
            nc.sync.dma_start(out=biasr[:], in_=d_bias[:])

            niota = cst[:, 0:1]

            # ---------- prep: y = fix(clip(ybf + offy)) in [64k x S] tiles
            prep = {}
            for side, (d_off, d_bg) in enumerate(
                ((d_offy, d_ybf), (d_offx, d_xbf))
            ):
                for t in range(NT):
                    r = min(64, K - 64 * t)
                    raw = wp.tile([r, S], f32, tag="raw", name=f"raw{side}{t}")
                    nc.sync.dma_start(out=raw[:], in_=d_off[64 * t : 64 * t + r, :])
                    bg = wp.tile([r, S], f32, tag="bg", name=f"bg{side}{t}")
                    nc.sync.dma_start(out=bg[:], in_=d_bg[64 * t : 64 * t + r, :])
                    yt = pp.tile([r, S], f32, tag=f"prep{side}{t}", name=f"prep{side}{t}")
                    nc.vector.tensor_tensor(out=yt[:], in0=raw[:], in1=bg[:], op=Alu.add)
                    nc.vector.tensor_scalar(
                        out=yt[:], in0=yt[:], scalar1=0.0, scalar2=47.0,
                        op0=Alu.max, op1=Alu.min,
                    )
                    ee = wp.tile([r, S], f32, tag="ee", name=f"ee{side}{t}")
                    nc.vector.tensor_scalar(
                        out=ee[:], in0=yt[:], scalar1=47.0, scalar2=BIG,
                        op0=Alu.is_equal, op1=Alu.mult,
                    )
                    nc.vector.tensor_tensor(out=yt[:], in0=yt[:], in1=ee[:], op=Alu.add)
                    prep[(side, t)] = yt

            # combined copy tiles per k-block b: quadrants 0,1 = y rows,
            # quadrants 2,3 = x rows -- one stream_shuffle then replicates
            # y into rows 0-63 and x into rows 64-127 for any k in block.
            qt = {}
            for b in range(NB):
                r = min(32, K - 32 * b)
                t, q = divmod(b, 2)
                ysrc = prep[(0, t)][32 * q : 32 * q + r, :]
                xsrc = prep[(1, t)][32 * q : 32 * q + r, :]
                dst = pp.tile([128, S], f32, tag=f"q{b}", name=f"q{b}")
                nc.sync.dma_start(out=dst[0:r, :], in_=ysrc)
                nc.sync.dma_start(out=dst[32 : 32 + r, :], in_=ysrc)
                nc.sync.dma_start(out=dst[64 : 64 + r, :], in_=xsrc)
                nc.sync.dma_start(out=dst[96 : 96 + r, :], in_=xsrc)
                qt[b] = dst

            # ---------- G build: G48[y_win, k*48+c] ----------
            g48 = pp.tile([DY, K * 48], f32, tag="g48", name="g48")
            for c in range(48):
                gp = psG.tile([DY, K], f32, tag="gp", name=f"gp{c}")
                nc.tensor.matmul(
                    out=gp[:], lhsT=mtw[:, c::48], rhs=kct[:],
                    start=True, stop=True,
                )
                nc.scalar.copy(out=g48[:, c::48], in_=gp[:])

            # ---------- main loop ----------
            SH = S // 2
            for u in range(U):
                acc = [psB.tile([1, S], f32, tag="acc", name=f"acc_{u}")]
                for ij in range(9):
                    k = ij * U + u
                    b, ksub = divmod(k, 32)

                    rep = wp.tile([128, S], f32, tag="rep", name=f"rep_{u}_{ij}")
                    msk = [ksub] * 32
                    nc.vector.stream_shuffle(
                        out=rep[:], in_=qt[b][:], mask=msk
                    )
                    tnt = wp.tile([128, S], f32, tag="tnt", name=f"tnt_{u}_{ij}")
                    nc.scalar.activation(
                        out=tnt[:], in_=rep[:], func=Act.Abs,
                        bias=niota[0:128, :], scale=1.0,
                    )
                    nc.scalar.activation(
                        out=tnt[:], in_=tnt[:], func=Act.Relu, bias=1.0, scale=-1.0
                    )
                    ry = tnt
                    cx_off = 64

                    ta = psA.tile([48, S], f32, tag="ta", name=f"ta_{u}_{ij}")
                    for lo, hi in ((0, 512), (512, S)):
                        nc.tensor.matmul(
                            out=ta[:, lo:hi],
                            lhsT=g48[:, 48 * k : 48 * (k + 1)],
                            rhs=ry[0:48, lo:hi],
                            start=True, stop=True,
                        )

                    pt_ = wp.tile([48, S], f32, tag="pt", name=f"pt_{u}_{ij}")
                    nc.vector.tensor_tensor(
                        out=pt_[:],
                        in0=ta[:],
                        in1=ry[cx_off : cx_off + 48, :],
                        op=Alu.mult,
                    )

                    for lo, hi in ((0, 512), (512, S)):
                        nc.tensor.matmul(
                            out=acc[0][:, lo:hi],
                            lhsT=ones48[:],
                            rhs=pt_[:, lo:hi],
                            start=(ij == 0), stop=(ij == 8),
                        )

                ot = wp.tile([1, S], f32, tag="ot", name=f"ot_{u}")
                nc.scalar.activation(
                    out=ot[:],
                    in_=acc[0][:],
                    func=Act.Identity,
                    bias=biasr[:, u : u + 1],
                    scale=1.0,
                )
                nc.sync.dma_start(out=d_out[:, u * S : (u + 1) * S], in_=ot[:])

    nc.compile()
    return nc


def _r0_for(h0):
    return min(max(h0 - 8, 0), H - DY)


def kernel(inputs, offset, kernel, bias):
    from concourse.bass_utils import run_bass_kernel_spmd

    inputs = np.asarray(inputs, np.float32)
    offset = np.asarray(offset, np.float32)
    kernel = np.asarray(kernel, np.float32)
    bias = np.asarray(bias, np.float32)

    if "nc" not in _PROG:
        _PROG["nc"] = _build_program()
    nc = _PROG["nc"]

    yb9, xb9 = _base_grids()
    k4 = kernel.reshape(9, U, C).reshape(K, C)
    kct = np.ascontiguousarray(k4.T)
    ones48 = np.ones((48, 1), np.float32)
    biasr = np.ascontiguousarray(bias.reshape(1, U))

    in_maps = []
    for core in range(NCORES):
        bb, hc = divmod(core, 4)
        h0 = hc * HLOC
        xpad = np.pad(inputs[bb], ((PAD, PAD), (PAD, PAD), (0, 0)))[:H, :W]
        mtw = np.ascontiguousarray(xpad.reshape(H * W, C).T)      # [32, 2304]
        osl = offset[bb, h0 : h0 + HLOC].reshape(S, K, 2)
        offy = np.ascontiguousarray(osl[:, :, 0].T)
        offx = np.ascontiguousarray(osl[:, :, 1].T)
        yb_s = yb9[h0 : h0 + HLOC].reshape(S, 9).T                # [9, S]
        xb_s = xb9[h0 : h0 + HLOC].reshape(S, 9).T
        ybf = np.ascontiguousarray(np.repeat(yb_s, U, axis=0))    # [216, S]
        xbf = np.ascontiguousarray(np.repeat(xb_s, U, axis=0))
        cstm = np.full((128, 1), 1.0e4, np.float32)
        cstm[0:48, 0] = -np.arange(48)
        cstm[64:112, 0] = -np.arange(48)
        in_maps.append(
            dict(mtw=mtw, kct=kct, offy=offy, offx=offx, ybf=ybf, xbf=xbf,
                 cst=cstm, ones48=ones48, biasr=biasr)
        )

    import os as _os
    _trace = bool(int(_os.environ.get("KERNEL_TRACE", "0")))
    res = run_bass_kernel_spmd(
        nc, in_maps, list(range(NCORES)), trace=_trace)
    _PROG["last_results"] = res

    out = np.empty((B, H, W, U), np.float32)
    for core in range(NCORES):
        bb, hc = divmod(core, 4)
        h0 = hc * HLOC
        o = res.results[core]["out"].reshape(U, HLOC, W)
        out[bb, h0 : h0 + HLOC] = o.transpose(1, 2, 0)
    return out



# revision 3
# speedup vs baseline: 1.3217x; 1.3217x over previous
"""Deformable-conv layer kernel for 8 Trainium2 NeuronCores (Bass/Tile).

kernel(**inputs): full inputs -> full output [2,48,48,24] f32.
Data parallel over (batch, H/4) -> 8 shards of 576 pixels.

Per core (576 pixels s, 216 sample-channels k):
  G[y,k,c]  = sum_ch M[y*48+c, ch] * k4[k, ch]       (PE 1x1-conv compress)
  one stream_shuffle replicates y_k to rows 0-63 and x_k to rows 64-127,
  one Abs + one Relu build both tent (bilinear-weight) matrices:
  Ry[r, s]  = tent(y_k[s] - r), Cx[c, s] = tent(x_k[s] - c)
  T_k       = G_k^T @ Ry   -> [48c x 576s]           (PE, N=512+64 split)
  P_k       = T_k * Cx_k                             (DVE, one op)
  out_u    += ones^T @ P_k  (partition sum + 9-tap PSUM accumulation)
tent(d) = relu(1-|d|) equals the reference bilinear weights exactly; the
y==47 / x==47 clip corner (reference weights all zero there) is handled
by adding 1e6 to the coordinate so the tents vanish.
"""

import sys

for _p in ("/opt/trn_rl_repo",):
    if _p not in sys.path:
        sys.path.insert(0, _p)

import numpy as np

B, H, W, C = 2, 48, 48, 32
U = 24
KH = KW = 3
PAD = 1
K = KH * KW * U          # 216
NCORES = 8
HLOC = H // 4            # 12
S = HLOC * W             # 576
DY = 48                  # y domain (full; border samples have base 0)
DX = 48                  # x domain
BIG = 1.0e6

_PROG = {}


def _base_grids():
    hh = np.arange(H)[:, None, None, None] + np.arange(KH)[None, None, :, None] - PAD
    ww = np.arange(W)[None, :, None, None] + np.arange(KW)[None, None, None, :] - PAD
    hh = np.broadcast_to(hh, (H, W, KH, KW))
    ww = np.broadcast_to(ww, (H, W, KH, KW))
    valid = (hh >= 0) & (hh < H) & (ww >= 0) & (ww < W)
    yb = np.where(valid, hh, 0).reshape(H, W, 9).astype(np.float32)
    xb = np.where(valid, ww, 0).reshape(H, W, 9).astype(np.float32)
    return yb, xb


def _build_program():
    import concourse.mybir as mybir
    import concourse.tile as tile
    from concourse import bacc

    f32 = mybir.dt.float32
    f32r = mybir.dt.float32r
    Alu = mybir.AluOpType
    Act = mybir.ActivationFunctionType

    nc = bacc.Bacc("TRN2", target_bir_lowering=False, debug=False)

    d_mtw = nc.declare_dram_parameter("mtw", [C, H * W], f32, isOutput=False)
    d_kct = nc.declare_dram_parameter("kct", [C, K], f32, isOutput=False)
    d_offy = nc.declare_dram_parameter("offy", [K, S], f32, isOutput=False)
    d_offx = nc.declare_dram_parameter("offx", [K, S], f32, isOutput=False)
    d_ybf = nc.declare_dram_parameter("ybf", [K, S], f32, isOutput=False)
    d_xbf = nc.declare_dram_parameter("xbf", [K, S], f32, isOutput=False)
    # consts [128, 1]: -iota (rows 0-63; junk above 48 is harmless)
    d_cst = nc.declare_dram_parameter("cst", [128, 1], f32, isOutput=False)
    d_ones = nc.declare_dram_parameter("ones48", [48, 1], f32, isOutput=False)
    d_bias = nc.declare_dram_parameter("biasr", [1, U], f32, isOutput=False)
    d_out = nc.declare_dram_parameter("out", [1, U * S], f32, isOutput=True)

    NB = (K + 31) // 32                 # 7 blocks of 32 k
    NT = (K + 63) // 64                 # 4 prep tiles of 64 k

    with tile.TileContext(nc) as tc:
        with (
            tc.tile_pool(name="persist", bufs=1) as pp,
            tc.tile_pool(name="work", bufs=3) as wp,
            tc.tile_pool(name="psA", bufs=2, space="PSUM") as psA,
            tc.tile_pool(name="psB", bufs=1, space="PSUM") as psB,
            tc.tile_pool(name="psG", bufs=1, space="PSUM") as psG,
        ):
            mtw0 = pp.tile([C, H * W], f32, tag="mtw0", name="mtw0")
            nc.sync.dma_start(out=mtw0[:], in_=d_mtw[:])
            kct0 = pp.tile([C, K], f32, tag="kct0", name="kct0")
            nc.sync.dma_start(out=kct0[:], in_=d_kct[:])
            # PE operands go through DVE copies so Matmult instructions
            # wait on engine semaphores only (walrus chokes on multiple
            # DMA-lane waits attached to one LdWeights).
            mtw = pp.tile([C, H * W], f32, tag="mtw", name="mtw")
            nc.vector.tensor_copy(out=mtw[:], in_=mtw0[:])
            kct = pp.tile([C, K], f32, tag="kct", name="kct")
            nc.vector.tensor_copy(out=kct[:], in_=kct0[:])
            cst = pp.tile([128, 1], f32, tag="cst", name="cst")
            nc.sync.dma_start(out=cst[:], in_=d_cst[:])
            ones0 = pp.tile([48, 1], f32, tag="ones0", name="ones0")
            nc.sync.dma_start(out=ones0[:], in_=d_ones[:])
            ones48 = pp.tile([48, 1], f32, tag="ones48", name="ones48")
            nc.vector.tensor_copy(out=ones48[:], in_=ones0[:])
            biasr = pp.tile([1, U], f32, tag="biasr", name="biasr")
            nc.sync.dma_start(out=biasr[:], in_=d_bias[:])

            niota = cst[:, 0:1]

            # ---------- prep: y = fix(clip(ybf + offy)) in [64k x S] tiles
            prep = {}
            for side, (d_off, d_bg) in enumerate(
                ((d_offy, d_ybf), (d_offx, d_xbf))
            ):
                for t in range(NT):
                    r = min(64, K - 64 * t)
                    raw = wp.tile([r, S], f32, tag="raw", name=f"raw{side}{t}")
                    nc.sync.dma_start(out=raw[:], in_=d_off[64 * t : 64 * t + r, :])
                    bg = wp.tile([r, S], f32, tag="bg", name=f"bg{side}{t}")
                    nc.sync.dma_start(out=bg[:], in_=d_bg[64 * t : 64 * t + r, :])
                    yt = pp.tile([r, S], f32, tag=f"prep{side}{t}", name=f"prep{side}{t}")
                    nc.vector.tensor_tensor(out=yt[:], in0=raw[:], in1=bg[:], op=Alu.add)
                    nc.vector.tensor_scalar(
                        out=yt[:], in0=yt[:], scalar1=0.0, scalar2=47.0,
                        op0=Alu.max, op1=Alu.min,
                    )
                    ee = wp.tile([r, S], f32, tag="ee", name=f"ee{side}{t}")
                    nc.vector.tensor_scalar(
                        out=ee[:], in0=yt[:], scalar1=47.0, scalar2=BIG,
                        op0=Alu.is_equal, op1=Alu.mult,
                    )
                    nc.vector.tensor_tensor(out=yt[:], in0=yt[:], in1=ee[:], op=Alu.add)
                    prep[(side, t)] = yt

            # combined copy tiles per k-block b: quadrants 0,1 = y rows,
            # quadrants 2,3 = x rows -- one stream_shuffle then replicates
            # y into rows 0-63 and x into rows 64-127 for any k in block.
            qt = {}
            for b in range(NB):
                r = min(32, K - 32 * b)
                t, q = divmod(b, 2)
                ysrc = prep[(0, t)][32 * q : 32 * q + r, :]
                xsrc = prep[(1, t)][32 * q : 32 * q + r, :]
                dst = pp.tile([128, S], f32, tag=f"q{b}", name=f"q{b}")
                nc.sync.dma_start(out=dst[0:r, :], in_=ysrc)
                nc.sync.dma_start(out=dst[32 : 32 + r, :], in_=ysrc)
                nc.sync.dma_start(out=dst[64 : 64 + r, :], in_=xsrc)
                nc.sync.dma_start(out=dst[96 : 96 + r, :], in_=xsrc)
                qt[b] = dst

            # ---------- G build: G48[y_win, k*48+c] ----------
            g48 = pp.tile([DY, K * 48], f32, tag="g48", name="g48")
            for c in range(48):
                gp = psG.tile([DY, K], f32, tag="gp", name=f"gp{c}")
                nc.tensor.matmul(
                    out=gp[:], lhsT=mtw[:, c::48].bitcast(f32r), rhs=kct[:].bitcast(f32r),
                    start=True, stop=True,
                )
                nc.scalar.copy(out=g48[:, c::48], in_=gp[:])

            # ---------- main loop ----------
            SH = S // 2
            for u in range(U):
                acc = [psB.tile([1, S], f32, tag="acc", name=f"acc_{u}")]
                for ij in range(9):
                    k = ij * U + u
                    b, ksub = divmod(k, 32)

                    rep = wp.tile([128, S], f32, tag="rep", name=f"rep_{u}_{ij}")
                    msk = [ksub] * 32
                    nc.vector.stream_shuffle(
                        out=rep[:], in_=qt[b][:], mask=msk
                    )
                    tnt = wp.tile([128, S], f32, tag="tnt", name=f"tnt_{u}_{ij}")
                    nc.scalar.activation(
                        out=tnt[:], in_=rep[:], func=Act.Abs,
                        bias=niota[0:128, :], scale=1.0,
                    )
                    nc.scalar.activation(
                        out=tnt[:], in_=tnt[:], func=Act.Relu, bias=1.0, scale=-1.0
                    )
                    ry = tnt
                    cx_off = 64

                    ta = psA.tile([48, S], f32, tag="ta", name=f"ta_{u}_{ij}")
                    for lo, hi in ((0, 512), (512, S)):
                        nc.tensor.matmul(
                            out=ta[:, lo:hi],
                            lhsT=g48[:, 48 * k : 48 * (k + 1)].bitcast(f32r),
                            rhs=ry[0:48, lo:hi].bitcast(f32r),
                            start=True, stop=True,
                        )

                    pt_ = wp.tile([48, S], f32, tag="pt", name=f"pt_{u}_{ij}")
                    nc.vector.tensor_tensor(
                        out=pt_[:],
                        in0=ta[:],
                        in1=ry[cx_off : cx_off + 48, :],
                        op=Alu.mult,
                    )

                    for lo, hi in ((0, 512), (512, S)):
                        nc.tensor.matmul(
                            out=acc[0][:, lo:hi],
                            lhsT=ones48[:].bitcast(f32r),
                            rhs=pt_[:, lo:hi].bitcast(f32r),
                            start=(ij == 0), stop=(ij == 8),
                        )

                ot = wp.tile([1, S], f32, tag="ot", name=f"ot_{u}")
                nc.scalar.activation(
                    out=ot[:],
                    in_=acc[0][:],
                    func=Act.Identity,
                    bias=biasr[:, u : u + 1],
                    scale=1.0,
                )
                nc.sync.dma_start(out=d_out[:, u * S : (u + 1) * S], in_=ot[:])

    nc.compile()
    return nc


def _r0_for(h0):
    return min(max(h0 - 8, 0), H - DY)


def kernel(inputs, offset, kernel, bias):
    from concourse.bass_utils import run_bass_kernel_spmd

    inputs = np.asarray(inputs, np.float32)
    offset = np.asarray(offset, np.float32)
    kernel = np.asarray(kernel, np.float32)
    bias = np.asarray(bias, np.float32)

    if "nc" not in _PROG:
        _PROG["nc"] = _build_program()
    nc = _PROG["nc"]

    yb9, xb9 = _base_grids()
    k4 = kernel.reshape(9, U, C).reshape(K, C)
    kct = np.ascontiguousarray(k4.T)
    ones48 = np.ones((48, 1), np.float32)
    biasr = np.ascontiguousarray(bias.reshape(1, U))

    in_maps = []
    for core in range(NCORES):
        bb, hc = divmod(core, 4)
        h0 = hc * HLOC
        xpad = np.pad(inputs[bb], ((PAD, PAD), (PAD, PAD), (0, 0)))[:H, :W]
        mtw = np.ascontiguousarray(xpad.reshape(H * W, C).T)      # [32, 2304]
        osl = offset[bb, h0 : h0 + HLOC].reshape(S, K, 2)
        offy = np.ascontiguousarray(osl[:, :, 0].T)
        offx = np.ascontiguousarray(osl[:, :, 1].T)
        yb_s = yb9[h0 : h0 + HLOC].reshape(S, 9).T                # [9, S]
        xb_s = xb9[h0 : h0 + HLOC].reshape(S, 9).T
        ybf = np.ascontiguousarray(np.repeat(yb_s, U, axis=0))    # [216, S]
        xbf = np.ascontiguousarray(np.repeat(xb_s, U, axis=0))
        cstm = np.full((128, 1), 1.0e4, np.float32)
        cstm[0:48, 0] = -np.arange(48)
        cstm[64:112, 0] = -np.arange(48)
        in_maps.append(
            dict(mtw=mtw, kct=kct, offy=offy, offx=offx, ybf=ybf, xbf=xbf,
                 cst=cstm, ones48=ones48, biasr=biasr)
        )

    import os as _os
    _trace = bool(int(_os.environ.get("KERNEL_TRACE", "0")))
    res = run_bass_kernel_spmd(
        nc, in_maps, list(range(NCORES)), trace=_trace)
    _PROG["last_results"] = res

    out = np.empty((B, H, W, U), np.float32)
    for core in range(NCORES):
        bb, hc = divmod(core, 4)
        h0 = hc * HLOC
        o = res.results[core]["out"].reshape(U, HLOC, W)
        out[bb, h0 : h0 + HLOC] = o.transpose(1, 2, 0)
    return out



# revision 15
# speedup vs baseline: 2.0246x; 1.5318x over previous
"""Deformable-conv layer kernel for 8 Trainium2 NeuronCores (Bass/Tile), v2.

kernel(**inputs): full inputs -> full output [2,48,48,24] f32.
Data parallel over (batch, H/4) -> 8 shards of 576 pixels.

Per core (576 pixels s, 216 sample-channels k = 9 taps x 24 groups):
  G[y, k*48+c] = sum_ch I'[y*48+c, ch] * k4[k, ch]       (PE, fp32r)
  rep = d[r, s]: PE "broadcast" matmul, contraction over 5 coordinate
    component rows (yi, yf, xi, xf, 1) -> d[r,s] = coord[s] - r for
    y rows 0-47 / x rows 48-95 (integer/fraction split keeps fp32r exact)
  tnt = +-tent(d) via Abs+Relu (Act) or Abs + tensor_scalar min (DVE/Pool)
  T_k  = G_k^T @ tnt[y rows]  -> [48c x s]               (PE, fp32r)
  P_k  = T_k * tnt[x rows]                               (DVE/Pool)
  acc[2u] += ones2^T @ P-pair  (u-paired partition sum, 9-tap PSUM accum)
tent sign flips per-iteration depending on engine (Act: +, min-trick: -);
P = (+-ty)*(+-tx) is always positive. The y==47 / x==47 clip corner
(reference weights all zero) is handled with a +1e6 coordinate offset.
"""

import sys

for _p in ("/opt/trn_rl_repo",):
    if _p not in sys.path:
        sys.path.insert(0, _p)

import numpy as np

B, H, W, C = 2, 48, 48, 32
U = 24
KH = KW = 3
PAD = 1
K = KH * KW * U          # 216
NCORES = 8
HLOC = H // 4            # 12
S = HLOC * W             # 576
BIG = 1.0e6
SC = 288                 # psum chunk (2 chunks of 288 = 576)

_PROG = {}


# per-iteration engine assignment knobs (keyed on emission index so the
# mix is uniform within every pair)
def _t_engine(i):
    r = i % 20
    return "act" if r in (1, 8, 15) else "pool"


def _m_engine(i):
    return "dve"


def _base_grids():
    hh = np.arange(H)[:, None, None, None] + np.arange(KH)[None, None, :, None] - PAD
    ww = np.arange(W)[None, :, None, None] + np.arange(KW)[None, None, None, :] - PAD
    hh = np.broadcast_to(hh, (H, W, KH, KW))
    ww = np.broadcast_to(ww, (H, W, KH, KW))
    valid = (hh >= 0) & (hh < H) & (ww >= 0) & (ww < W)
    yb = np.where(valid, hh, 0).reshape(H, W, 9).astype(np.float32)
    xb = np.where(valid, ww, 0).reshape(H, W, 9).astype(np.float32)
    return yb, xb


def _build_program():
    import concourse.mybir as mybir
    import concourse.tile as tile
    from concourse import bacc

    f32 = mybir.dt.float32
    f32r = mybir.dt.float32r
    Alu = mybir.AluOpType
    Act = mybir.ActivationFunctionType

    nc = bacc.Bacc("TRN2", target_bir_lowering=False, debug=False)

    d_mtw = nc.declare_dram_parameter("mtw", [C, H * W], f32, isOutput=False)
    d_kct = nc.declare_dram_parameter("kct", [C, 256], f32, isOutput=False)
    d_cyx = [
        nc.declare_dram_parameter(f"cyx{t}", [5, 18 * S], f32, isOutput=False)
        for t in range(U // 2)
    ]
    d_sel = nc.declare_dram_parameter("sel5", [5, 113], f32, isOutput=False)
    d_gsum = nc.declare_dram_parameter("gsumneg", [1, K * 48], f32, isOutput=False)
    d_ones2 = nc.declare_dram_parameter("ones2", [112, 2], f32, isOutput=False)
    d_bias = nc.declare_dram_parameter("biasr", [2, U // 2], f32, isOutput=False)
    d_out = nc.declare_dram_parameter("out", [2, (U // 2) * S], f32, isOutput=True)

    with tile.TileContext(nc) as tc:
        with (
            tc.tile_pool(name="persist", bufs=1) as pp,
            tc.tile_pool(name="coord", bufs=2) as cp,
            tc.tile_pool(name="work", bufs=6) as wp,
            tc.tile_pool(name="psR", bufs=2, space="PSUM") as psR,
            tc.tile_pool(name="psA", bufs=2, space="PSUM") as psA,
            tc.tile_pool(name="psB", bufs=1, space="PSUM") as psB,
        ):
            # ---- constant loads + fp32r-rounding copies for PE operands ----
            mtw0 = pp.tile([C, H * W], f32, tag="mtw0", name="mtw0")
            nc.sync.dma_start(out=mtw0[:], in_=d_mtw[:])
            mtw = pp.tile([C, H * W], f32, tag="mtw", name="mtw")
            nc.vector.tensor_copy(out=mtw[:].bitcast(f32r), in_=mtw0[:])
            kct0 = pp.tile([C, 256], f32, tag="kct0", name="kct0")
            nc.sync.dma_start(out=kct0[:], in_=d_kct[:])
            kct = pp.tile([C, 256], f32, tag="kct", name="kct")
            nc.vector.tensor_copy(out=kct[:].bitcast(f32r), in_=kct0[:])
            sel0 = pp.tile([5, 113], f32, tag="sel0", name="sel0")
            nc.sync.dma_start(out=sel0[:], in_=d_sel[:])
            sel5 = pp.tile([5, 113], f32, tag="sel5", name="sel5")
            nc.vector.tensor_copy(out=sel5[:].bitcast(f32r), in_=sel0[:])
            on0 = pp.tile([112, 2], f32, tag="on0", name="on0")
            nc.sync.dma_start(out=on0[:], in_=d_ones2[:])
            ones2 = pp.tile([112, 2], f32, tag="ones2", name="ones2")
            nc.vector.tensor_copy(out=ones2[:].bitcast(f32r), in_=on0[:])
            biasr = pp.tile([2, U // 2], f32, tag="biasr", name="biasr")
            nc.sync.dma_start(out=biasr[:], in_=d_bias[:])

            # ---- G build: G[y, k*48+c]; row 48 = -colsum(G) ----
            g48 = pp.tile([49, K * 48], f32, tag="g48", name="g48")
            nc.sync.dma_start(
                out=g48[48:49, :].bitcast(f32r), in_=d_gsum[:].bitcast(f32r)
            )
            for c in range(48):
                gp = psA.tile([48, SC], f32, tag="ta", name=f"gp{c}")
                nc.tensor.matmul(
                    out=gp[:, 0:256],
                    lhsT=mtw[:, c::48].bitcast(f32r),
                    rhs=kct[:].bitcast(f32r),
                    start=True, stop=True,
                )
                geng = (nc.vector, nc.scalar)[c % 2]
                if geng is nc.scalar:
                    nc.scalar.copy(out=g48[0:48, c::48].bitcast(f32r),
                                   in_=gp[:, 0:K])
                else:
                    geng.tensor_copy(out=g48[0:48, c::48].bitcast(f32r),
                                     in_=gp[:, 0:K])

            # ---- main loop: 12 u-pairs x 9 taps x 2 in-pair ----
            out24 = pp.tile([2, (U // 2) * S], f32, tag="out24", name="out24")
            CH = ((0, 0), (SC, 512))      # (data offset, acc offset)
            for p in range(U // 2):
                cyx = cp.tile([5, 18 * S], f32r, tag="cyx", name=f"cyx{p}")
                nc.sync.dma_start(out=cyx[:], in_=d_cyx[p][:].bitcast(f32r))
                acc = psB.tile([2, 1024], f32, tag="acc", name=f"acc_{p}")
                for ij in range(9):
                    pt2 = wp.tile([112, S], f32, tag="pt2", name=f"pt2_{p}_{ij}")
                    if p * 9 + ij < 6:
                        nc.vector.memset(pt2[32:64, :], 0.0)
                    for uu in range(2):
                        k = ij * U + 2 * p + uu
                        kb = (ij * 2 + uu) * S
                        idx = p * 18 + ij * 2 + uu
                        te = _t_engine(idx)

                        tnt = wp.tile([113, S], f32, tag="tnt", name=f"tnt_{k}")
                        rep = psR.tile([113, 2, 512], f32, tag="rep",
                                       name=f"rep_{k}")
                        for ci, (lo, _) in enumerate(CH):
                            nc.tensor.matmul(
                                out=rep[:, ci, 0:SC],
                                lhsT=sel5[:].bitcast(f32r),
                                rhs=cyx[:, kb + lo : kb + lo + SC],
                                start=True, stop=True,
                            )
                        # |d| in one op; tent (act) / -tent (pool) below
                        nc.scalar.activation(
                            out=tnt[:, 0:S].bitcast(f32r),
                            in_=rep[:, :, 0:SC],
                            func=Act.Abs, bias=0.0, scale=1.0,
                        )
                        if te == "act":
                            nc.scalar.activation(
                                out=tnt[:].bitcast(f32r), in_=tnt[:],
                                func=Act.Relu, bias=1.0, scale=-1.0,
                            )
                        elif te == "pool":
                            # -tent = min(|d|-1, 0); row 48 -> 0, and the
                            # two sign flips cancel in P (SBUF-only op)
                            nc.gpsimd.tensor_scalar(
                                out=tnt[:].bitcast(f32r), in0=tnt[:],
                                scalar1=1.0, scalar2=0.0,
                                op0=Alu.subtract, op1=Alu.min,
                            )

                        for lo, _ in CH:
                            ta = psA.tile([48, SC], f32, tag="ta",
                                          name=f"ta_{k}_{lo}")
                            nc.tensor.matmul(
                                out=ta[:],
                                lhsT=g48[:, 48 * k : 48 * (k + 1)].bitcast(f32r),
                                rhs=tnt[0:49, lo : lo + SC].bitcast(f32r),
                                start=True, stop=True,
                            )
                            dst = pt2[64 * uu : 64 * uu + 48,
                                      lo : lo + SC].bitcast(f32r)
                            nc.vector.tensor_tensor(
                                out=dst, in0=ta[:],
                                in1=tnt[64:112, lo : lo + SC],
                                op=Alu.mult,
                            )
                    for lo, ao in CH:
                        nc.tensor.matmul(
                            out=acc[:, ao : ao + SC],
                            lhsT=ones2[:].bitcast(f32r),
                            rhs=pt2[:, lo : lo + SC].bitcast(f32r),
                            start=(ij == 0), stop=(ij == 8),
                            skip_group_check=True,
                        )
                for lo, ao in CH:
                    nc.scalar.activation(
                        out=out24[:, p * S + lo : p * S + lo + SC],
                        in_=acc[:, ao : ao + SC],
                        func=Act.Identity, bias=biasr[:, p : p + 1],
                        scale=1.0,
                    )
            nc.sync.dma_start(out=d_out[:], in_=out24[:])

    nc.compile()
    return nc


def kernel(inputs, offset, kernel, bias):
    from concourse.bass_utils import run_bass_kernel_spmd

    inputs = np.asarray(inputs, np.float32)
    offset = np.asarray(offset, np.float32)
    kernel = np.asarray(kernel, np.float32)
    bias = np.asarray(bias, np.float32)

    if "nc" not in _PROG:
        _PROG["nc"] = _build_program()
    nc = _PROG["nc"]

    yb9, xb9 = _base_grids()
    k4 = kernel.reshape(9, U, C).reshape(K, C)
    kct = np.zeros((C, 256), np.float32)
    kct[:, 0:K] = k4.T

    sel5 = np.zeros((5, 113), np.float32)
    r = np.arange(113)
    sel5[0, :] = (r < 48)                  # yi
    sel5[1, :] = (r < 48)                  # yf
    sel5[2, :] = (r >= 64)                 # xi
    sel5[3, :] = (r >= 64)                 # xf
    sel5[4, :] = -np.where(r < 48, r, np.maximum(r - 64, 0)).astype(np.float32)
    sel5[4, 48] = 9.0                      # d[48] = 9 -> row48: min-path 1, tent-path 0
    sel5[4, 49:64] = 9.0                   # unused rows, keep tents there 0/1
    ones2 = np.zeros((112, 2), np.float32)
    ones2[0:48, 0] = 1.0
    ones2[64:112, 1] = 1.0
    biasr = np.ascontiguousarray(bias.reshape(U // 2, 2).T)

    in_maps = []
    for core in range(NCORES):
        bb, hc = divmod(core, 4)
        h0 = hc * HLOC
        xpad = np.pad(inputs[bb], ((PAD, PAD), (PAD, PAD), (0, 0)))[:H, :W]
        mtw = np.ascontiguousarray(xpad.reshape(H * W, C).T)      # [32, 2304]
        # -colsum of G per (k, c): G[r, 48k+c] = sum_ch I'[r,c,ch] k4[k,ch]
        colsum = xpad.sum(axis=0).astype(np.float32)              # [48, 32]
        gsum = k4 @ colsum.T                                      # [216k, 48c]
        gsumneg = np.ascontiguousarray(-gsum.reshape(1, K * 48))
        osl = offset[bb, h0 : h0 + HLOC].reshape(S, K, 2)
        yc = yb9[h0 : h0 + HLOC].reshape(S, 9)
        xc = xb9[h0 : h0 + HLOC].reshape(S, 9)
        y = np.clip(np.repeat(yc, U, axis=1) + osl[:, :, 0], 0.0, 47.0)  # [S,K]
        x = np.clip(np.repeat(xc, U, axis=1) + osl[:, :, 1], 0.0, 47.0)
        y = y + BIG * (y == 47.0)
        x = x + BIG * (x == 47.0)
        yi = np.floor(y)
        xi = np.floor(x)
        yf = (y - yi).astype(np.float32)
        xf = (x - xi).astype(np.float32)
        comp = (yi.T.astype(np.float32), yf.T, xi.T.astype(np.float32), xf.T)
        im = {"mtw": mtw, "kct": kct, "sel5": sel5, "ones2": ones2,
              "biasr": biasr, "gsumneg": gsumneg}
        for t in range(U // 2):
            blk = np.ones((5, 18 * S), np.float32)
            for ij in range(9):
                for uu in range(2):
                    k = ij * U + 2 * t + uu
                    col = (ij * 2 + uu) * S
                    for rr in range(4):
                        blk[rr, col : col + S] = comp[rr][k]
            im[f"cyx{t}"] = blk
        in_maps.append(im)

    import os as _os
    _trace = bool(int(_os.environ.get("KERNEL_TRACE", "0")))
    res = run_bass_kernel_spmd(
        nc, in_maps, list(range(NCORES)), trace=_trace)
    _PROG["last_results"] = res

    out = np.empty((B, H, W, U), np.float32)
    for core in range(NCORES):
        bb, hc = divmod(core, 4)
        h0 = hc * HLOC
        o = res.results[core]["out"].reshape(2, U // 2, HLOC, W)
        out[bb, h0 : h0 + HLOC] = o.transpose(2, 3, 1, 0).reshape(HLOC, W, U)
    return out


# revision 16
# speedup vs baseline: 2.0428x; 1.0090x over previous
"""Deformable-conv layer kernel for 8 Trainium2 NeuronCores (Bass/Tile), v2.

kernel(**inputs): full inputs -> full output [2,48,48,24] f32.
Data parallel over (batch, H/4) -> 8 shards of 576 pixels.

Per core (576 pixels s, 216 sample-channels k = 9 taps x 24 groups):
  G[y, k*48+c] = sum_ch I'[y*48+c, ch] * k4[k, ch]       (PE, fp32r)
  rep = d[r, s]: PE "broadcast" matmul, contraction over 5 coordinate
    component rows (yi, yf, xi, xf, 1) -> d[r,s] = coord[s] - r for
    y rows 0-47 / x rows 48-95 (integer/fraction split keeps fp32r exact)
  tnt = +-tent(d) via Abs+Relu (Act) or Abs + tensor_scalar min (DVE/Pool)
  T_k  = G_k^T @ tnt[y rows]  -> [48c x s]               (PE, fp32r)
  P_k  = T_k * tnt[x rows]                               (DVE/Pool)
  acc[2u] += ones2^T @ P-pair  (u-paired partition sum, 9-tap PSUM accum)
tent sign flips per-iteration depending on engine (Act: +, min-trick: -);
P = (+-ty)*(+-tx) is always positive. The y==47 / x==47 clip corner
(reference weights all zero) is handled with a +1e6 coordinate offset.
"""

import sys

for _p in ("/opt/trn_rl_repo",):
    if _p not in sys.path:
        sys.path.insert(0, _p)

import numpy as np

B, H, W, C = 2, 48, 48, 32
U = 24
KH = KW = 3
PAD = 1
K = KH * KW * U          # 216
NCORES = 8
HLOC = H // 4            # 12
S = HLOC * W             # 576
BIG = 1.0e6
SC = 288                 # psum chunk (2 chunks of 288 = 576)

_PROG = {}


# per-iteration engine assignment knobs (keyed on emission index so the
# mix is uniform within every pair)
def _t_engine(i):
    r = i % 20
    return "act" if r in (1, 11) else "pool"


def _m_engine(i):
    return "dve"


def _base_grids():
    hh = np.arange(H)[:, None, None, None] + np.arange(KH)[None, None, :, None] - PAD
    ww = np.arange(W)[None, :, None, None] + np.arange(KW)[None, None, None, :] - PAD
    hh = np.broadcast_to(hh, (H, W, KH, KW))
    ww = np.broadcast_to(ww, (H, W, KH, KW))
    valid = (hh >= 0) & (hh < H) & (ww >= 0) & (ww < W)
    yb = np.where(valid, hh, 0).reshape(H, W, 9).astype(np.float32)
    xb = np.where(valid, ww, 0).reshape(H, W, 9).astype(np.float32)
    return yb, xb


def _build_program():
    import concourse.mybir as mybir
    import concourse.tile as tile
    from concourse import bacc

    f32 = mybir.dt.float32
    f32r = mybir.dt.float32r
    Alu = mybir.AluOpType
    Act = mybir.ActivationFunctionType

    nc = bacc.Bacc("TRN2", target_bir_lowering=False, debug=False)

    d_mtw = nc.declare_dram_parameter("mtw", [C, H * W], f32, isOutput=False)
    d_kct = nc.declare_dram_parameter("kct", [C, 256], f32, isOutput=False)
    d_cyx = [
        nc.declare_dram_parameter(f"cyx{t}", [5, 18 * S], f32, isOutput=False)
        for t in range(U // 2)
    ]
    d_sel = nc.declare_dram_parameter("sel5", [5, 113], f32, isOutput=False)
    d_gsum = nc.declare_dram_parameter("gsumneg", [1, K * 48], f32, isOutput=False)
    d_ones2 = nc.declare_dram_parameter("ones2", [112, 2], f32, isOutput=False)
    d_bias = nc.declare_dram_parameter("biasr", [2, U // 2], f32, isOutput=False)
    d_out = nc.declare_dram_parameter("out", [2, (U // 2) * S], f32, isOutput=True)

    with tile.TileContext(nc) as tc:
        with (
            tc.tile_pool(name="persist", bufs=1) as pp,
            tc.tile_pool(name="coord", bufs=2) as cp,
            tc.tile_pool(name="work", bufs=6) as wp,
            tc.tile_pool(name="psR", bufs=2, space="PSUM") as psR,
            tc.tile_pool(name="psA", bufs=2, space="PSUM") as psA,
            tc.tile_pool(name="psB", bufs=1, space="PSUM") as psB,
        ):
            # ---- constant loads + fp32r-rounding copies for PE operands ----
            mtw0 = pp.tile([C, H * W], f32, tag="mtw0", name="mtw0")
            nc.sync.dma_start(out=mtw0[:], in_=d_mtw[:])
            mtw = pp.tile([C, H * W], f32, tag="mtw", name="mtw")
            nc.vector.tensor_copy(out=mtw[:].bitcast(f32r), in_=mtw0[:])
            kct0 = pp.tile([C, 256], f32, tag="kct0", name="kct0")
            nc.sync.dma_start(out=kct0[:], in_=d_kct[:])
            kct = pp.tile([C, 256], f32, tag="kct", name="kct")
            nc.vector.tensor_copy(out=kct[:].bitcast(f32r), in_=kct0[:])
            sel0 = pp.tile([5, 113], f32, tag="sel0", name="sel0")
            nc.sync.dma_start(out=sel0[:], in_=d_sel[:])
            sel5 = pp.tile([5, 113], f32, tag="sel5", name="sel5")
            nc.vector.tensor_copy(out=sel5[:].bitcast(f32r), in_=sel0[:])
            on0 = pp.tile([112, 2], f32, tag="on0", name="on0")
            nc.sync.dma_start(out=on0[:], in_=d_ones2[:])
            ones2 = pp.tile([112, 2], f32, tag="ones2", name="ones2")
            nc.vector.tensor_copy(out=ones2[:].bitcast(f32r), in_=on0[:])
            biasr = pp.tile([2, U // 2], f32, tag="biasr", name="biasr")
            nc.sync.dma_start(out=biasr[:], in_=d_bias[:])

            # ---- G build: G[y, k*48+c]; row 48 = -colsum(G) ----
            g48 = pp.tile([49, K * 48], f32, tag="g48", name="g48")
            nc.sync.dma_start(
                out=g48[48:49, :].bitcast(f32r), in_=d_gsum[:].bitcast(f32r)
            )
            for c in range(48):
                gp = psA.tile([48, SC], f32, tag="ta", name=f"gp{c}")
                nc.tensor.matmul(
                    out=gp[:, 0:256],
                    lhsT=mtw[:, c::48].bitcast(f32r),
                    rhs=kct[:].bitcast(f32r),
                    start=True, stop=True,
                )
                geng = (nc.vector, nc.scalar)[c % 2]
                if geng is nc.scalar:
                    nc.scalar.copy(out=g48[0:48, c::48].bitcast(f32r),
                                   in_=gp[:, 0:K])
                else:
                    geng.tensor_copy(out=g48[0:48, c::48].bitcast(f32r),
                                     in_=gp[:, 0:K])

            # ---- main loop: 12 u-pairs x 9 taps x 2 in-pair ----
            out24 = pp.tile([2, (U // 2) * S], f32, tag="out24", name="out24")
            CH = ((0, 0), (SC, 512))      # (data offset, acc offset)
            for p in range(U // 2):
                cyx = cp.tile([5, 18 * S], f32r, tag="cyx", name=f"cyx{p}")
                nc.sync.dma_start(out=cyx[:], in_=d_cyx[p][:].bitcast(f32r))
                acc = psB.tile([2, 1024], f32, tag="acc", name=f"acc_{p}")
                for ij in range(9):
                    pt2 = wp.tile([112, S], f32, tag="pt2", name=f"pt2_{p}_{ij}")
                    if p * 9 + ij < 6:
                        nc.scalar.memzero(pt2[32:64, :])
                    for uu in range(2):
                        k = ij * U + 2 * p + uu
                        kb = (ij * 2 + uu) * S
                        idx = p * 18 + ij * 2 + uu
                        te = _t_engine(idx)

                        tnt = wp.tile([113, S], f32, tag="tnt", name=f"tnt_{k}")
                        rep = psR.tile([113, 2, 512], f32, tag="rep",
                                       name=f"rep_{k}")
                        for ci, (lo, _) in enumerate(CH):
                            nc.tensor.matmul(
                                out=rep[:, ci, 0:SC],
                                lhsT=sel5[:].bitcast(f32r),
                                rhs=cyx[:, kb + lo : kb + lo + SC],
                                start=True, stop=True,
                            )
                        # |d| in one op; tent (act) / -tent (pool) below
                        nc.scalar.activation(
                            out=tnt[:, 0:S].bitcast(f32r),
                            in_=rep[:, :, 0:SC],
                            func=Act.Abs, bias=0.0, scale=1.0,
                        )
                        if te == "act":
                            nc.scalar.activation(
                                out=tnt[:].bitcast(f32r), in_=tnt[:],
                                func=Act.Relu, bias=1.0, scale=-1.0,
                            )
                        elif te == "pool":
                            # -tent = min(|d|-1, 0); row 48 -> 0, and the
                            # two sign flips cancel in P (SBUF-only op)
                            nc.gpsimd.tensor_scalar(
                                out=tnt[:].bitcast(f32r), in0=tnt[:],
                                scalar1=1.0, scalar2=0.0,
                                op0=Alu.subtract, op1=Alu.min,
                            )

                        for lo, _ in CH:
                            ta = psA.tile([48, SC], f32, tag="ta",
                                          name=f"ta_{k}_{lo}")
                            nc.tensor.matmul(
                                out=ta[:],
                                lhsT=g48[:, 48 * k : 48 * (k + 1)].bitcast(f32r),
                                rhs=tnt[0:49, lo : lo + SC].bitcast(f32r),
                                start=True, stop=True,
                            )
                            dst = pt2[64 * uu : 64 * uu + 48,
                                      lo : lo + SC].bitcast(f32r)
                            nc.vector.tensor_tensor(
                                out=dst, in0=ta[:],
                                in1=tnt[64:112, lo : lo + SC],
                                op=Alu.mult,
                            )
                    for lo, ao in CH:
                        nc.tensor.matmul(
                            out=acc[:, ao : ao + SC],
                            lhsT=ones2[:].bitcast(f32r),
                            rhs=pt2[:, lo : lo + SC].bitcast(f32r),
                            start=(ij == 0), stop=(ij == 8),
                            skip_group_check=True,
                        )
                for lo, ao in CH:
                    nc.scalar.activation(
                        out=out24[:, p * S + lo : p * S + lo + SC],
                        in_=acc[:, ao : ao + SC],
                        func=Act.Identity, bias=biasr[:, p : p + 1],
                        scale=1.0,
                    )
            nc.sync.dma_start(out=d_out[:], in_=out24[:])

    nc.compile()
    return nc


def kernel(inputs, offset, kernel, bias):
    from concourse.bass_utils import run_bass_kernel_spmd

    inputs = np.asarray(inputs, np.float32)
    offset = np.asarray(offset, np.float32)
    kernel = np.asarray(kernel, np.float32)
    bias = np.asarray(bias, np.float32)

    if "nc" not in _PROG:
        _PROG["nc"] = _build_program()
    nc = _PROG["nc"]

    yb9, xb9 = _base_grids()
    k4 = kernel.reshape(9, U, C).reshape(K, C)
    kct = np.zeros((C, 256), np.float32)
    kct[:, 0:K] = k4.T

    sel5 = np.zeros((5, 113), np.float32)
    r = np.arange(113)
    sel5[0, :] = (r < 48)                  # yi
    sel5[1, :] = (r < 48)                  # yf
    sel5[2, :] = (r >= 64)                 # xi
    sel5[3, :] = (r >= 64)                 # xf
    sel5[4, :] = -np.where(r < 48, r, np.maximum(r - 64, 0)).astype(np.float32)
    sel5[4, 48] = 9.0                      # d[48] = 9 -> row48: min-path 1, tent-path 0
    sel5[4, 49:64] = 9.0                   # unused rows, keep tents there 0/1
    ones2 = np.zeros((112, 2), np.float32)
    ones2[0:48, 0] = 1.0
    ones2[64:112, 1] = 1.0
    biasr = np.ascontiguousarray(bias.reshape(U // 2, 2).T)

    in_maps = []
    for core in range(NCORES):
        bb, hc = divmod(core, 4)
        h0 = hc * HLOC
        xpad = np.pad(inputs[bb], ((PAD, PAD), (PAD, PAD), (0, 0)))[:H, :W]
        mtw = np.ascontiguousarray(xpad.reshape(H * W, C).T)      # [32, 2304]
        # -colsum of G per (k, c): G[r, 48k+c] = sum_ch I'[r,c,ch] k4[k,ch]
        colsum = xpad.sum(axis=0).astype(np.float32)              # [48, 32]
        gsum = k4 @ colsum.T                                      # [216k, 48c]
        gsumneg = np.ascontiguousarray(-gsum.reshape(1, K * 48))
        osl = offset[bb, h0 : h0 + HLOC].reshape(S, K, 2)
        yc = yb9[h0 : h0 + HLOC].reshape(S, 9)
        xc = xb9[h0 : h0 + HLOC].reshape(S, 9)
        y = np.clip(np.repeat(yc, U, axis=1) + osl[:, :, 0], 0.0, 47.0)  # [S,K]
        x = np.clip(np.repeat(xc, U, axis=1) + osl[:, :, 1], 0.0, 47.0)
        y = y + BIG * (y == 47.0)
        x = x + BIG * (x == 47.0)
        yi = np.floor(y)
        xi = np.floor(x)
        yf = (y - yi).astype(np.float32)
        xf = (x - xi).astype(np.float32)
        comp = (yi.T.astype(np.float32), yf.T, xi.T.astype(np.float32), xf.T)
        im = {"mtw": mtw, "kct": kct, "sel5": sel5, "ones2": ones2,
              "biasr": biasr, "gsumneg": gsumneg}
        for t in range(U // 2):
            blk = np.ones((5, 18 * S), np.float32)
            for ij in range(9):
                for uu in range(2):
                    k = ij * U + 2 * t + uu
                    col = (ij * 2 + uu) * S
                    for rr in range(4):
                        blk[rr, col : col + S] = comp[rr][k]
            im[f"cyx{t}"] = blk
        in_maps.append(im)

    import os as _os
    _trace = bool(int(_os.environ.get("KERNEL_TRACE", "0")))
    res = run_bass_kernel_spmd(
        nc, in_maps, list(range(NCORES)), trace=_trace)
    _PROG["last_results"] = res

    out = np.empty((B, H, W, U), np.float32)
    for core in range(NCORES):
        bb, hc = divmod(core, 4)
        h0 = hc * HLOC
        o = res.results[core]["out"].reshape(2, U // 2, HLOC, W)
        out[bb, h0 : h0 + HLOC] = o.transpose(2, 3, 1, 0).reshape(HLOC, W, U)
    return out
